# revision 46
# baseline (speedup 1.0000x reference)
"""Trainium2 Bass kernel for the scan-RNN problem (B=2048, T=512, H=256).

Data-parallel over batch: 8 cores x 256 rows each. The T=512 recurrence runs
fully on-chip per core; weights are replicated.

Reference math (per step, gamma/beta fold away since they are 1/0):
    z   = (h + tanh(x_t*W_embed + b_embed)) @ W' + b'
    u   = tanh(z)
    h   = (u - mean(u)) * rsqrt(var(u) + eps)

Deferred-layernorm formulation: the on-chip state is the UNNORMALIZED u,
kept transposed (uT[j, b], bf16). Normalization folds into the next step:
    h@W' = rstd*(u@W') + d*c,   d = -mean*rstd,  c = colsum(W')
so each step runs
    pv = uT @ W'                      (PE, 2 K-chunks)
    z  = rstd_prev * pv + w1          (DVE STT, PSUM+SBUF)
    w1 = d_prev * c_rep + G[x_t]+b'   (DVE STT, precomputed off-chain)
    u  = tanh(z)                      (ACT, accum -> sum)
    uT = transpose(u)                 (PE transpose-mode, bf16)
with G[x_t] rows DMA-gathered from a host-precomputed [T,B,H] table, so the
PE only runs the 4 W-matmuls + 4 transposes per step and the entire stats
chain (sumsq via one fused STT-accum, magic-rsqrt with a single Newton step)
runs off the critical path, overlapped with the next step's matmuls.

The stats algebra is folded to skip mean and eps entirely:
    ve' = H^2*var = sqsum*H - usum^2  (exact power-of-two scale)
    rsqrt(var) seed magic becomes MAGIC + 4*2^23 (exponent shift by H=2^8)
    d = (usum * -1/H) * rstd

Everything the PE touches is bf16 (1 cycle/row vs fp32's 4); PSUM and stats
stay fp32. Host-sim puts end-to-end rel err ~7e-3, inside the 2e-2 gate.
"""

import numpy as np

H = 256
EPS = 1e-5  # dropped on device: ve' >= H^2*var >> H^2*eps in practice
NCORES = 8
NV = 10  # x values are 0..9
GB = 8   # G-table steps per DMA batch
GPRE = 4  # prefetch lead (steps) for the G-table DMA

MAGIC = 0x5F3759DF + 4 * (1 << 23)  # rsqrt seed magic, pre-shifted for ve'=H^2*var

# blob column layout (all bf16, 128 partitions)
_WP0 = 0           # W' chunk 0 [128, 256]
_WP1 = 256         # W' chunk 1 [128, 256]
_ID = 512          # identity [128, 128]
_C2 = 640          # -colsum(W')/H in rows 0 and 1 [2, 256]
_WO = 896          # W_out' chunks [128, 2, 16]
_COREP = 928       # -colsum(W_out')/H replicated [128, 16]
_BOREP = 944       # b_out' replicated [128, 16]
_ZERO = 960        # zeros [128, 512] = u0 transposed state
_CW = 960 + 512    # blob width


def build_nc(T, B_local):
    """Build the Bass program for one core (SPMD: all cores identical)."""
    import concourse.bass as bass
    import concourse.mybir as mybir
    import concourse.tile as tile
    from concourse import bacc

    dt = mybir.dt
    AF = mybir.ActivationFunctionType
    OP = mybir.AluOpType
    nc = bacc.Bacc(None, target_bir_lowering=False, debug=False)

    NB = B_local // 128  # batch half-tiles (2)
    assert B_local % 128 == 0 and NB == 2

    nbat = (T + GB - 1) // GB
    gs = nc.declare_dram_parameter(
        "gs", [nbat, 128, GB, NB, H], dt.bfloat16, isOutput=False)
    cst = nc.declare_dram_parameter("cst", [128, _CW], dt.bfloat16,
                                    isOutput=False)
    out = nc.declare_dram_parameter("out", [B_local, 16], dt.float32,
                                    isOutput=True)

    with tile.TileContext(nc) as tc:
        with (
            tc.tile_pool(name="singles", bufs=1) as singles,
            tc.tile_pool(name="gpool", bufs=3) as gpool,
            tc.tile_pool(name="state", bufs=2) as state,
            tc.tile_pool(name="work", bufs=2) as work,
            tc.tile_pool(name="stats", bufs=2) as stats,
            tc.tile_pool(name="psum_v", bufs=2, space="PSUM") as psum_v,
            tc.tile_pool(name="psum_t", bufs=1, space="PSUM") as psum_t,
        ):
            # ---- one DMA for every constant -----------------------------
            blob = singles.tile([128, _CW], dt.bfloat16, tag="blob")
            nc.sync.dma_start(out=blob, in_=cst[:, :])
            wp0 = blob[:, _WP0:_WP0 + H]
            wp1 = blob[:, _WP1:_WP1 + H]
            ident = blob[:, _ID:_ID + 128]
            c2 = blob[:, _C2:_C2 + H]  # rows 0/1 hold -colsum(W')/H
            wo_sb = blob[:, _WO:_WO + 32].rearrange("p (c h) -> p c h", c=2)
            co_rep = blob[:, _COREP:_COREP + 16]
            bo_rep = blob[:, _BOREP:_BOREP + 16]
            u0 = blob[:, _ZERO:_ZERO + 2 * B_local].rearrange(
                "p (c b) -> p c b", c=2)

            uTs = [u0[:, :, 0:128], u0[:, :, 128:256]]
            mdrow_prev = None   # [2, 128] bf16: md per half, row layout
            gts = [None] * nbat

            def load_gbatch(i):
                gt = gpool.tile([128, GB, NB, H], dt.bfloat16, tag="gt")
                nc.sync.dma_start(out=gt, in_=gs[i, :, :, :, :])
                gts[i] = gt

            load_gbatch(0)
            if nbat > 1:
                load_gbatch(1)

            rstd_prev = None
            md_prev = None
            for t in range(T):
                tn = t + GPRE
                if tn % GB == 0:
                    i = tn // GB
                    if 2 <= i < nbat and gts[i] is None:
                        load_gbatch(i)

                gslice = gts[t // GB][:, t % GB, :, :]  # [128, NB, H]

                # ---- matmuls + fixup + tanh, both halves ----------------
                us = []
                sums = stats.tile([128, 2, NB], dt.float32, tag="sums")
                usum = sums[:, 0, :]
                sqsum = sums[:, 1, :]
                for hb in range(NB):
                    u = work.tile([128, H], dt.bfloat16, tag=f"u{hb}")
                    if t == 0:
                        # u0 state is zero: z = G[x_0] + b' directly
                        nc.scalar.activation(
                            u, gslice[:, hb, :], AF.Tanh,
                            accum_out=usum[:, hb:hb + 1],
                        )
                    else:
                        # pv = uT@W' + md_prev (x) c'  (the deferred-norm
                        # mean term enters as a rank-1 K=1 matmul)
                        pv = psum_v.tile([128, H], dt.float32, tag=f"pv{hb}")
                        nc.tensor.matmul(pv, lhsT=uTs[hb][:, 0, :], rhs=wp0,
                                         start=True, stop=False)
                        nc.tensor.matmul(pv, lhsT=uTs[hb][:, 1, :], rhs=wp1,
                                         start=False, stop=False)
                        nc.tensor.matmul(
                            pv, lhsT=mdrow_prev[32 * hb:32 * hb + 1, :],
                            rhs=c2[32 * hb:32 * hb + 1, :],
                            start=False, stop=True)
                        z = work.tile([128, H], dt.bfloat16, tag=f"z{hb}")
                        nc.vector.scalar_tensor_tensor(
                            out=z, in0=pv, scalar=rstd_prev[:, hb:hb + 1],
                            in1=gslice[:, hb, :], op0=OP.mult, op1=OP.add,
                        )
                        nc.scalar.activation(
                            u, z, AF.Tanh, accum_out=usum[:, hb:hb + 1],
                        )
                    us.append(u)

                # ---- sumsq per half: one fused square+accumulate --------
                scr = work.tile([128, NB, H], dt.bfloat16, tag="scr")
                for hb in range(NB):
                    nc.vector.scalar_tensor_tensor(
                        out=scr[:, hb, :], in0=us[hb], scalar=1.0,
                        in1=us[hb], op0=OP.mult, op1=OP.mult,
                        accum_out=sqsum[:, hb:hb + 1],
                    )

                # ---- transpose both halves; evacuate both on ACT --------
                # (keeps the DVE stream free for the stats chain, which
                # gates the next step's fixup)
                pts = []
                new_uTs = []
                for hb in range(NB):
                    pt = psum_t.tile([128, 2, 128], dt.bfloat16, tag=f"pt{hb}")
                    for c in range(2):
                        nc.tensor.transpose(
                            out=pt[:, c, :], in_=us[hb][:, bass.ts(c, 128)],
                            identity=ident,
                        )
                    pts.append(pt)
                    uT = state.tile([128, 2, 128], dt.bfloat16, tag=f"uT{hb}")
                    new_uTs.append(uT)
                nc.scalar.copy(out=new_uTs[0], in_=pts[0])
                nc.scalar.copy(out=new_uTs[1], in_=pts[1])
                uTs = new_uTs

                # ---- usum into row layout for the next K=1 matmul -------
                # z = rstd*(u@W' + usum (x) c') + g with c' = -c/H gives
                # exactly the -mean*rstd*colsum(W') deferred-norm term, so
                # the rank-1 row is plain usum: no stats dependency at all.
                usb = stats.tile([128, NB], dt.bfloat16, tag="usb")
                nc.gpsimd.tensor_copy(out=usb, in_=usum)
                pmd = psum_t.tile([128, 2, 128], dt.bfloat16, tag="pmd")
                for hb in range(NB):
                    nc.tensor.transpose(out=pmd[0:1, hb, :],
                                        in_=usb[:, hb:hb + 1],
                                        identity=ident)
                mdrow = stats.tile([64, 128], dt.bfloat16, tag="mdrow")
                nc.scalar.copy(out=mdrow[0:1, :], in_=pmd[0:1, 0, :])
                nc.scalar.copy(out=mdrow[32:33, :], in_=pmd[0:1, 1, :])
                mdrow_prev = mdrow

                # ---- stats chain on [128, NB] columns -------------------
                # ve' = H*var = sqsum - usum^2/H; eps dropped (negligible)
                sc = stats.tile([128, 5, NB], dt.float32, tag="sc")
                m2u = sc[:, 0, :]
                ve = sc[:, 1, :]
                y0 = sc[:, 2, :]
                tt = sc[:, 3, :]
                rstd = sc[:, 4, :]
                nc.gpsimd.tensor_tensor(out=m2u, in0=usum, in1=usum,
                                        op=OP.mult)
                nc.vector.scalar_tensor_tensor(
                    out=ve, in0=m2u, scalar=-1.0 / H, in1=sqsum,
                    op0=OP.mult, op1=OP.add,
                )
                # seed: float(bits(ve')) -> linear -> int -> bits as float
                # (MAGIC pre-shifted by 4*2^23 for the H=2^8 scale of ve')
                nc.vector.tensor_copy(out=y0, in_=ve.bitcast(dt.int32))
                nc.vector.tensor_scalar(
                    out=y0, in0=y0, scalar1=-0.5, scalar2=float(MAGIC),
                    op0=OP.mult, op1=OP.add,
                )
                nc.vector.tensor_copy(out=y0.bitcast(dt.int32), in_=y0)
                # one Newton step: rstd = y*(1.5 - 0.5*(ve'/H)*y^2)
                nc.gpsimd.tensor_tensor(out=tt, in0=y0, in1=y0, op=OP.mult)
                nc.vector.scalar_tensor_tensor(
                    out=tt, in0=ve, scalar=-0.5 / H, in1=tt,
                    op0=OP.mult, op1=OP.mult,
                )
                nc.vector.scalar_tensor_tensor(
                    out=rstd, in0=tt, scalar=1.5, in1=y0,
                    op0=OP.add, op1=OP.mult,
                )
                rstd_prev = rstd

            # ---- final projection: out = rstd*(u@Wo' + usum (x) co') + bo'
            po = psum_t.tile([128, NB, 16], dt.float32, tag="po")
            ot = work.tile([128, NB, 16], dt.float32, tag="ot")
            for hb in range(NB):
                nc.tensor.matmul(
                    po[:, hb, :], lhsT=uTs[hb][:, 0, :], rhs=wo_sb[:, 0, :],
                    start=True, stop=False,
                )
                nc.tensor.matmul(
                    po[:, hb, :], lhsT=uTs[hb][:, 1, :], rhs=wo_sb[:, 1, :],
                    start=False, stop=False,
                )
                nc.tensor.matmul(
                    po[:, hb, :], lhsT=mdrow_prev[32 * hb:32 * hb + 1, :],
                    rhs=co_rep[32 * hb:32 * hb + 1, :],
                    start=False, stop=True,
                )
                nc.vector.scalar_tensor_tensor(
                    out=ot[:, hb, :], in0=po[:, hb, :],
                    scalar=rstd_prev[:, hb:hb + 1], in1=bo_rep[:, :],
                    op0=OP.mult, op1=OP.add,
                )
            nc.sync.dma_start(
                out=out[:, :].rearrange("(c p) h -> p c h", p=128), in_=ot
            )

    nc.finalize()
    return nc


def _prepare_host(x, W_embed, b_embed, W_update, b_update, gamma, beta,
                  W_out, b_out):
    """Fold gamma/beta into the weights; build the G table + consts blob."""
    import ml_dtypes

    Wp = (gamma[:, None] * W_update).astype(np.float32)  # [H, H]
    bp = (b_update + beta @ W_update).astype(np.float32)  # [H]
    Wo = (gamma[:, None] * W_out).astype(np.float32)  # [H, 10]
    bo = (b_out + beta @ W_out).astype(np.float32)  # [10]

    vals = np.arange(NV, dtype=np.float32)[:, None]
    E = np.tanh(vals @ W_embed + b_embed).astype(np.float32)  # [10, H]
    Grow = (E @ Wp + bp).astype(np.float32)  # [10, H]: z-contribution per x

    cst = np.zeros((128, _CW), np.float32)
    cst[:, _WP0:_WP0 + H] = Wp[0:128]
    cst[:, _WP1:_WP1 + H] = Wp[128:256]
    cst[:, _ID:_ID + 128] = np.eye(128, dtype=np.float32)
    cst[:, _C2:_C2 + H] = (-1.0 / H) * Wp.sum(axis=0)[None, :]
    cst[:, _WO:_WO + 16] = np.pad(Wo[0:128], ((0, 0), (0, 6)))
    cst[:, _WO + 16:_WO + 32] = np.pad(Wo[128:256], ((0, 0), (0, 6)))
    cst[:, _COREP:_COREP + 16] = (-1.0 / H) * np.pad(Wo.sum(axis=0), (0, 6))[None, :]
    cst[:, _BOREP:_BOREP + 16] = np.pad(bo, (0, 6))[None, :]
    # _ZERO region stays zero = u0
    return Grow, cst.astype(ml_dtypes.bfloat16)


def prepare(x, W_embed, b_embed, W_update, b_update, gamma, beta, W_out, b_out,
            T_override=None, B_override=None):
    import ml_dtypes

    x = np.asarray(x, np.float32)
    B = x.shape[0] if B_override is None else B_override
    T = x.shape[1] if T_override is None else T_override
    x = x[:B, :T]

    Grow, cst = _prepare_host(
        np.asarray(x), np.asarray(W_embed), np.asarray(b_embed),
        np.asarray(W_update), np.asarray(b_update), np.asarray(gamma),
        np.asarray(beta), np.asarray(W_out), np.asarray(b_out),
    )
    Grow16 = Grow.astype(ml_dtypes.bfloat16)

    B_local = B // NCORES
    nc = build_nc(T, B_local)

    nbat = (T + GB - 1) // GB
    xi = x[:, :, 0].astype(np.int32)  # [B, T]
    in_maps = []
    for c in range(NCORES):
        xc = xi[c * B_local:(c + 1) * B_local]  # [256, T]
        # gs[i, p, g, hb, :] = Grow[x[hb*128+p, i*GB+g]]
        xcr = xc.reshape(2, 128, T).transpose(2, 0, 1)  # [T, hb, p]
        g = Grow16[xcr]  # [T, 2, 128, H]
        g = g.reshape(nbat, GB, 2, 128, H).transpose(0, 3, 1, 2, 4)
        in_maps.append({
            "gs": np.ascontiguousarray(g),
            "cst": cst,
        })
    return nc, in_maps


def _numpy_fallback(x, W_embed, b_embed, W_update, b_update, gamma, beta,
                    W_out, b_out):
    """Reference math on host; only for inputs the device kernel can't take
    (non-integer x or values outside 0..9 - never happens with the spec'd
    randint fill, but better safe than crashed)."""
    xb = x[:, :, 0]
    B, T = xb.shape
    h = np.zeros((B, H), np.float32)
    for t in range(T):
        inp = np.tanh(xb[:, t:t + 1] @ W_embed + b_embed)
        z = (inp + h) @ W_update + b_update
        u = np.tanh(z)
        mu = u.mean(-1, keepdims=True)
        var = ((u - mu) ** 2).mean(-1, keepdims=True)
        h = (u - mu) / np.sqrt(var + EPS) * gamma + beta
    return (h @ W_out + b_out).astype(np.float32)


def kernel(x, W_embed, b_embed, W_update, b_update, gamma, beta, W_out, b_out,
           T_override=None, B_override=None):
    x = np.asarray(x, np.float32)
    xi = x[:, :, 0]
    if not (np.all(xi == np.round(xi)) and xi.min() >= 0 and xi.max() < NV
            and x.shape[0] % (NCORES * 128) == 0):
        return _numpy_fallback(
            x, np.asarray(W_embed, np.float32), np.asarray(b_embed, np.float32),
            np.asarray(W_update, np.float32), np.asarray(b_update, np.float32),
            np.asarray(gamma, np.float32), np.asarray(beta, np.float32),
            np.asarray(W_out, np.float32), np.asarray(b_out, np.float32))

    nc, in_maps = prepare(x, W_embed, b_embed, W_update, b_update, gamma, beta,
                          W_out, b_out, T_override, B_override)

    from concourse.bass_utils import run_bass_kernel_spmd

    res = run_bass_kernel_spmd(nc, in_maps, list(range(NCORES)))
    global LAST_RESULT
    LAST_RESULT = res
    outs = [res.results[c]["out"][:, :10] for c in range(NCORES)]
    return np.concatenate(outs, axis=0).astype(np.float32)


LAST_RESULT = None


# revision 49
# speedup vs baseline: 1.0020x; 1.0020x over previous
"""Trainium2 Bass kernel for the scan-RNN problem (B=2048, T=512, H=256).

Data-parallel over batch: 8 cores x 256 rows each. The T=512 recurrence runs
fully on-chip per core; weights are replicated.

Reference math (per step, gamma/beta fold away since they are 1/0):
    z   = (h + tanh(x_t*W_embed + b_embed)) @ W' + b'
    u   = tanh(z)
    h   = (u - mean(u)) * rsqrt(var(u) + eps)

Deferred-layernorm formulation: the on-chip state is the UNNORMALIZED u,
kept transposed (uT[j, b], bf16). Normalization folds into the next step:
    h@W' = rstd*(u@W') + d*c,   d = -mean*rstd,  c = colsum(W')
so each step runs
    pv = uT @ W'                      (PE, 2 K-chunks)
    z  = rstd_prev * pv + w1          (DVE STT, PSUM+SBUF)
    w1 = d_prev * c_rep + G[x_t]+b'   (DVE STT, precomputed off-chain)
    u  = tanh(z)                      (ACT, accum -> sum)
    uT = transpose(u)                 (PE transpose-mode, bf16)
with G[x_t] rows DMA-gathered from a host-precomputed [T,B,H] table, so the
PE only runs the 4 W-matmuls + 4 transposes per step and the entire stats
chain (sumsq via one fused STT-accum, magic-rsqrt with a single Newton step)
runs off the critical path, overlapped with the next step's matmuls.

The stats algebra is folded to skip mean and eps entirely:
    ve' = H^2*var = sqsum*H - usum^2  (exact power-of-two scale)
    rsqrt(var) seed magic becomes MAGIC + 4*2^23 (exponent shift by H=2^8)
    d = (usum * -1/H) * rstd

Everything the PE touches is bf16 (1 cycle/row vs fp32's 4); PSUM and stats
stay fp32. Host-sim puts end-to-end rel err ~7e-3, inside the 2e-2 gate.
"""

import numpy as np

H = 256
EPS = 1e-5  # dropped on device: ve' >= H^2*var >> H^2*eps in practice
NCORES = 8
NV = 10  # x values are 0..9
GB = 8   # G-table steps per DMA batch
GPRE = 4  # prefetch lead (steps) for the G-table DMA

MAGIC = 0x5F3759DF + 4 * (1 << 23)  # rsqrt seed magic, pre-shifted for ve'=H^2*var

# blob column layout (all bf16, 128 partitions)
_WP0 = 0           # W' chunk 0 [128, 256]
_WP1 = 256         # W' chunk 1 [128, 256]
_ID = 512          # identity [128, 128]
_C2 = 640          # -colsum(W')/H in rows 0 and 1 [2, 256]
_WO = 896          # W_out' chunks [128, 2, 16]
_COREP = 928       # -colsum(W_out')/H replicated [128, 16]
_BOREP = 944       # b_out' replicated [128, 16]
_ZERO = 960        # zeros [128, 512] = u0 transposed state
_CW = 960 + 512    # blob width


def build_nc(T, B_local):
    """Build the Bass program for one core (SPMD: all cores identical)."""
    import concourse.bass as bass
    import concourse.mybir as mybir
    import concourse.tile as tile
    from concourse import bacc

    dt = mybir.dt
    AF = mybir.ActivationFunctionType
    OP = mybir.AluOpType
    nc = bacc.Bacc(None, target_bir_lowering=False, debug=False)

    NB = B_local // 128  # batch half-tiles (2)
    assert B_local % 128 == 0 and NB == 2

    nbat = (T + GB - 1) // GB
    gs = nc.declare_dram_parameter(
        "gs", [nbat, 128, GB, NB, H], dt.bfloat16, isOutput=False)
    cst = nc.declare_dram_parameter("cst", [128, _CW], dt.bfloat16,
                                    isOutput=False)
    out = nc.declare_dram_parameter("out", [B_local, 16], dt.float32,
                                    isOutput=True)

    with tile.TileContext(nc) as tc:
        with (
            tc.tile_pool(name="singles", bufs=1) as singles,
            tc.tile_pool(name="gpool", bufs=3) as gpool,
            tc.tile_pool(name="state", bufs=2) as state,
            tc.tile_pool(name="work", bufs=2) as work,
            tc.tile_pool(name="stats", bufs=2) as stats,
            tc.tile_pool(name="psum_v", bufs=2, space="PSUM") as psum_v,
            tc.tile_pool(name="psum_t", bufs=1, space="PSUM") as psum_t,
        ):
            # ---- one DMA for every constant -----------------------------
            blob = singles.tile([128, _CW], dt.bfloat16, tag="blob")
            nc.sync.dma_start(out=blob, in_=cst[:, :])
            wp0 = blob[:, _WP0:_WP0 + H]
            wp1 = blob[:, _WP1:_WP1 + H]
            ident = blob[:, _ID:_ID + 128]
            c2 = blob[:, _C2:_C2 + H]  # rows 0/1 hold -colsum(W')/H
            wo_sb = blob[:, _WO:_WO + 32].rearrange("p (c h) -> p c h", c=2)
            co_rep = blob[:, _COREP:_COREP + 16]
            bo_rep = blob[:, _BOREP:_BOREP + 16]
            u0 = blob[:, _ZERO:_ZERO + 2 * B_local].rearrange(
                "p (c b) -> p c b", c=2)

            uTs = [u0[:, :, 0:128], u0[:, :, 128:256]]
            mdrow_prev = None   # [2, 128] bf16: md per half, row layout
            gts = [None] * nbat

            def load_gbatch(i):
                gt = gpool.tile([128, GB, NB, H], dt.bfloat16, tag="gt")
                nc.sync.dma_start(out=gt, in_=gs[i, :, :, :, :])
                gts[i] = gt

            load_gbatch(0)
            if nbat > 1:
                load_gbatch(1)

            rstd_prev = None
            md_prev = None
            for t in range(T):
                tn = t + GPRE
                if tn % GB == 0:
                    i = tn // GB
                    if 2 <= i < nbat and gts[i] is None:
                        load_gbatch(i)

                gslice = gts[t // GB][:, t % GB, :, :]  # [128, NB, H]

                # ---- matmuls + fixup + tanh, both halves ----------------
                us = []
                sums = stats.tile([128, 2, NB], dt.float32, tag="sums")
                usum = sums[:, 0, :]
                sqsum = sums[:, 1, :]
                for hb in range(NB):
                    u = work.tile([128, H], dt.bfloat16, tag=f"u{hb}")
                    if t == 0:
                        # u0 state is zero: z = G[x_0] + b' directly
                        nc.scalar.activation(
                            u, gslice[:, hb, :], AF.Tanh,
                            accum_out=usum[:, hb:hb + 1],
                        )
                    else:
                        # pv = uT@W' + md_prev (x) c'  (the deferred-norm
                        # mean term enters as a rank-1 K=1 matmul)
                        pv = psum_v.tile([128, H], dt.float32, tag=f"pv{hb}")
                        nc.tensor.matmul(pv, lhsT=uTs[hb][:, 0, :], rhs=wp0,
                                         start=True, stop=False)
                        nc.tensor.matmul(pv, lhsT=uTs[hb][:, 1, :], rhs=wp1,
                                         start=False, stop=False)
                        nc.tensor.matmul(
                            pv, lhsT=mdrow_prev[0:1, hb, :],
                            rhs=c2[0:1, :],
                            start=False, stop=True)
                        z = work.tile([128, H], dt.bfloat16, tag=f"z{hb}")
                        nc.vector.scalar_tensor_tensor(
                            out=z, in0=pv, scalar=rstd_prev[:, hb:hb + 1],
                            in1=gslice[:, hb, :], op0=OP.mult, op1=OP.add,
                        )
                        nc.scalar.activation(
                            u, z, AF.Tanh, accum_out=usum[:, hb:hb + 1],
                        )
                    us.append(u)

                # ---- sumsq per half: one fused square+accumulate --------
                scr = work.tile([128, NB, H], dt.bfloat16, tag="scr")
                for hb in range(NB):
                    nc.vector.scalar_tensor_tensor(
                        out=scr[:, hb, :], in0=us[hb], scalar=1.0,
                        in1=us[hb], op0=OP.mult, op1=OP.mult,
                        accum_out=sqsum[:, hb:hb + 1],
                    )

                # ---- transpose both halves; evacuate both on ACT --------
                # (keeps the DVE stream free for the stats chain, which
                # gates the next step's fixup)
                pts = []
                new_uTs = []
                for hb in range(NB):
                    pt = psum_t.tile([128, 2, 128], dt.bfloat16, tag=f"pt{hb}")
                    for c in range(2):
                        nc.tensor.transpose(
                            out=pt[:, c, :], in_=us[hb][:, bass.ts(c, 128)],
                            identity=ident,
                        )
                    pts.append(pt)
                    uT = state.tile([128, 2, 128], dt.bfloat16, tag=f"uT{hb}")
                    new_uTs.append(uT)
                nc.scalar.copy(out=new_uTs[0], in_=pts[0])
                nc.scalar.copy(out=new_uTs[1], in_=pts[1])
                uTs = new_uTs

                # ---- usum into row layout for the next K=1 matmul -------
                # z = rstd*(u@W' + usum (x) c') + g with c' = -c/H gives
                # exactly the -mean*rstd*colsum(W') deferred-norm term, so
                # the rank-1 row is plain usum: no stats dependency at all.
                usb = stats.tile([128, NB], dt.bfloat16, tag="usb")
                nc.gpsimd.tensor_copy(out=usb, in_=usum)
                pmd = psum_t.tile([128, 2, 128], dt.bfloat16, tag="pmd")
                for hb in range(NB):
                    nc.tensor.transpose(out=pmd[0:1, hb, :],
                                        in_=usb[:, hb:hb + 1],
                                        identity=ident)
                # both rows live on partition 0 at adjacent free ranges, so
                # one copy evacuates them and the K=1 matmuls slice by half
                mdrow = stats.tile([1, NB, 128], dt.bfloat16, tag="mdrow")
                nc.scalar.copy(out=mdrow, in_=pmd[0:1, :, :])
                mdrow_prev = mdrow

                # ---- stats chain on [128, NB] columns -------------------
                # ve' = H*var = sqsum - usum^2/H; eps dropped (negligible)
                sc = stats.tile([128, 5, NB], dt.float32, tag="sc")
                m2u = sc[:, 0, :]
                ve = sc[:, 1, :]
                y0 = sc[:, 2, :]
                tt = sc[:, 3, :]
                rstd = sc[:, 4, :]
                nc.gpsimd.tensor_tensor(out=m2u, in0=usum, in1=usum,
                                        op=OP.mult)
                nc.vector.scalar_tensor_tensor(
                    out=ve, in0=m2u, scalar=-1.0 / H, in1=sqsum,
                    op0=OP.mult, op1=OP.add,
                )
                # seed: float(bits(ve')) -> linear -> int -> bits as float
                # (MAGIC pre-shifted by 4*2^23 for the H=2^8 scale of ve')
                nc.vector.tensor_copy(out=y0, in_=ve.bitcast(dt.int32))
                nc.vector.tensor_scalar(
                    out=y0, in0=y0, scalar1=-0.5, scalar2=float(MAGIC),
                    op0=OP.mult, op1=OP.add,
                )
                nc.vector.tensor_copy(out=y0.bitcast(dt.int32), in_=y0)
                # one Newton step: rstd = y*(1.5 - 0.5*(ve'/H)*y^2)
                nc.gpsimd.tensor_tensor(out=tt, in0=y0, in1=y0, op=OP.mult)
                nc.vector.scalar_tensor_tensor(
                    out=tt, in0=ve, scalar=-0.5 / H, in1=tt,
                    op0=OP.mult, op1=OP.mult,
                )
                nc.vector.scalar_tensor_tensor(
                    out=rstd, in0=tt, scalar=1.5, in1=y0,
                    op0=OP.add, op1=OP.mult,
                )
                rstd_prev = rstd

            # ---- final projection: out = rstd*(u@Wo' + usum (x) co') + bo'
            po = psum_t.tile([128, NB, 16], dt.float32, tag="po")
            ot = work.tile([128, NB, 16], dt.float32, tag="ot")
            for hb in range(NB):
                nc.tensor.matmul(
                    po[:, hb, :], lhsT=uTs[hb][:, 0, :], rhs=wo_sb[:, 0, :],
                    start=True, stop=False,
                )
                nc.tensor.matmul(
                    po[:, hb, :], lhsT=uTs[hb][:, 1, :], rhs=wo_sb[:, 1, :],
                    start=False, stop=False,
                )
                nc.tensor.matmul(
                    po[:, hb, :], lhsT=mdrow_prev[0:1, hb, :],
                    rhs=co_rep[0:1, :],
                    start=False, stop=True,
                )
                nc.vector.scalar_tensor_tensor(
                    out=ot[:, hb, :], in0=po[:, hb, :],
                    scalar=rstd_prev[:, hb:hb + 1], in1=bo_rep[:, :],
                    op0=OP.mult, op1=OP.add,
                )
            nc.sync.dma_start(
                out=out[:, :].rearrange("(c p) h -> p c h", p=128), in_=ot
            )

    nc.finalize()
    return nc


def _prepare_host(x, W_embed, b_embed, W_update, b_update, gamma, beta,
                  W_out, b_out):
    """Fold gamma/beta into the weights; build the G table + consts blob."""
    import ml_dtypes

    Wp = (gamma[:, None] * W_update).astype(np.float32)  # [H, H]
    bp = (b_update + beta @ W_update).astype(np.float32)  # [H]
    Wo = (gamma[:, None] * W_out).astype(np.float32)  # [H, 10]
    bo = (b_out + beta @ W_out).astype(np.float32)  # [10]

    vals = np.arange(NV, dtype=np.float32)[:, None]
    E = np.tanh(vals @ W_embed + b_embed).astype(np.float32)  # [10, H]
    Grow = (E @ Wp + bp).astype(np.float32)  # [10, H]: z-contribution per x

    cst = np.zeros((128, _CW), np.float32)
    cst[:, _WP0:_WP0 + H] = Wp[0:128]
    cst[:, _WP1:_WP1 + H] = Wp[128:256]
    cst[:, _ID:_ID + 128] = np.eye(128, dtype=np.float32)
    cst[:, _C2:_C2 + H] = (-1.0 / H) * Wp.sum(axis=0)[None, :]
    cst[:, _WO:_WO + 16] = np.pad(Wo[0:128], ((0, 0), (0, 6)))
    cst[:, _WO + 16:_WO + 32] = np.pad(Wo[128:256], ((0, 0), (0, 6)))
    cst[:, _COREP:_COREP + 16] = (-1.0 / H) * np.pad(Wo.sum(axis=0), (0, 6))[None, :]
    cst[:, _BOREP:_BOREP + 16] = np.pad(bo, (0, 6))[None, :]
    # _ZERO region stays zero = u0
    return Grow, cst.astype(ml_dtypes.bfloat16)


def prepare(x, W_embed, b_embed, W_update, b_update, gamma, beta, W_out, b_out,
            T_override=None, B_override=None):
    import ml_dtypes

    x = np.asarray(x, np.float32)
    B = x.shape[0] if B_override is None else B_override
    T = x.shape[1] if T_override is None else T_override
    x = x[:B, :T]

    Grow, cst = _prepare_host(
        np.asarray(x), np.asarray(W_embed), np.asarray(b_embed),
        np.asarray(W_update), np.asarray(b_update), np.asarray(gamma),
        np.asarray(beta), np.asarray(W_out), np.asarray(b_out),
    )
    Grow16 = Grow.astype(ml_dtypes.bfloat16)

    B_local = B // NCORES
    nc = build_nc(T, B_local)

    nbat = (T + GB - 1) // GB
    xi = x[:, :, 0].astype(np.int32)  # [B, T]
    in_maps = []
    for c in range(NCORES):
        xc = xi[c * B_local:(c + 1) * B_local]  # [256, T]
        # gs[i, p, g, hb, :] = Grow[x[hb*128+p, i*GB+g]]
        xcr = xc.reshape(2, 128, T).transpose(2, 0, 1)  # [T, hb, p]
        g = Grow16[xcr]  # [T, 2, 128, H]
        g = g.reshape(nbat, GB, 2, 128, H).transpose(0, 3, 1, 2, 4)
        in_maps.append({
            "gs": np.ascontiguousarray(g),
            "cst": cst,
        })
    return nc, in_maps


def _numpy_fallback(x, W_embed, b_embed, W_update, b_update, gamma, beta,
                    W_out, b_out):
    """Reference math on host; only for inputs the device kernel can't take
    (non-integer x or values outside 0..9 - never happens with the spec'd
    randint fill, but better safe than crashed)."""
    xb = x[:, :, 0]
    B, T = xb.shape
    h = np.zeros((B, H), np.float32)
    for t in range(T):
        inp = np.tanh(xb[:, t:t + 1] @ W_embed + b_embed)
        z = (inp + h) @ W_update + b_update
        u = np.tanh(z)
        mu = u.mean(-1, keepdims=True)
        var = ((u - mu) ** 2).mean(-1, keepdims=True)
        h = (u - mu) / np.sqrt(var + EPS) * gamma + beta
    return (h @ W_out + b_out).astype(np.float32)


def kernel(x, W_embed, b_embed, W_update, b_update, gamma, beta, W_out, b_out,
           T_override=None, B_override=None):
    x = np.asarray(x, np.float32)
    xi = x[:, :, 0]
    if not (np.all(xi == np.round(xi)) and xi.min() >= 0 and xi.max() < NV
            and x.shape[0] % (NCORES * 128) == 0):
        return _numpy_fallback(
            x, np.asarray(W_embed, np.float32), np.asarray(b_embed, np.float32),
            np.asarray(W_update, np.float32), np.asarray(b_update, np.float32),
            np.asarray(gamma, np.float32), np.asarray(beta, np.float32),
            np.asarray(W_out, np.float32), np.asarray(b_out, np.float32))

    nc, in_maps = prepare(x, W_embed, b_embed, W_update, b_update, gamma, beta,
                          W_out, b_out, T_override, B_override)

    from concourse.bass_utils import run_bass_kernel_spmd

    res = run_bass_kernel_spmd(nc, in_maps, list(range(NCORES)))
    global LAST_RESULT
    LAST_RESULT = res
    outs = [res.results[c]["out"][:, :10] for c in range(NCORES)]
    return np.concatenate(outs, axis=0).astype(np.float32)


LAST_RESULT = None
